# revision 5
# baseline (speedup 1.0000x reference)
"""DeMash kernel for Trainium2 (8 NeuronCores, Bass/Tile).

Math: Y = X @ C^H over rows n = (B,T,S) flattened, with a subcarrier
gather before and scatter after. Complex multiply via the 3-matmul
Gauss trick (all operands f32r -> full PE rate, ~1.5e-4 rel err):
    m1 = Xr @ Cr^T,  m2 = Xi @ Ci^T,  m3 = (Xr+Xi) @ (Cr-Ci)^T
    Yr = m1 + m2,    Yi = m3 - m1 + m2
Sharding: data-parallel over batch (axis 0), 32 batches -> 256 rows per
core; C replicated. Stationary = X^T tiles, moving = C^T slabs, PSUM
accumulation over the L=1512 contraction in 12 tiles of 126. Xs and Cd
are formed on-device (DVE) to keep HBM traffic at the fp32 minimum.
"""

import numpy as np
import concourse.bass as bass
import concourse.mybir as mybir
from concourse import bacc
from concourse.tile import TileContext
from concourse.bass_utils import run_bass_kernel_spmd

B, T, S, SYM, FFT = 256, 4, 2, 14, 128
NSC = 108
L = SYM * NSC                   # 1512
NCORES = 8
ROWS = (B // NCORES) * T * S    # 256 rows per core
NT = ROWS // 128                # 2 row tiles of 128
KT, KP = 12, 126                # contraction tiles: 12 x 126 = 1512
NCH, NW = 3, 504                # output column chunks: 3 x 504 = 1512

F32 = mybir.dt.float32
F32R = mybir.dt.float32r

TRACE = False                   # test harness flips this for profiling
LAST_RESULTS = None             # stashed BassKernelResults for the harness

_NC = None


def _build_program():
    nc = bacc.Bacc("TRN2", target_bir_lowering=False, debug=False)
    XX = nc.dram_tensor("XX", [KT, KP, 2 * ROWS], F32R, kind="ExternalInput")
    CC = nc.dram_tensor("CC", [KT, NCH, KP, 2 * NW], F32R, kind="ExternalInput")
    YR = nc.dram_tensor("YR", [ROWS, L], F32, kind="ExternalOutput")
    YI = nc.dram_tensor("YI", [ROWS, L], F32, kind="ExternalOutput")

    with TileContext(nc) as tc:
        with (
            tc.tile_pool(name="xp", bufs=1) as xp,
            tc.tile_pool(name="cp", bufs=20) as cp,
            tc.tile_pool(name="dp", bufs=14) as dp,
            tc.tile_pool(name="op", bufs=6) as op,
            tc.tile_pool(name="pp", bufs=2, space="PSUM") as pp,
        ):
            # Resident stationary operands: X^T [126, (k, ri, n)] and
            # Xs = Xr+Xi [126, (k, n)].
            xt = xp.tile([128, KT * 2 * ROWS], F32R, tag="xt")
            xs = xp.tile([128, KT * ROWS], F32R, tag="xs")

            def xsl(k, ri, n):
                off = k * 2 * ROWS + ri * ROWS + n * 128
                return xt[:KP, off:off + 128]

            def xssl(k, n):
                off = k * ROWS + n * 128
                return xs[:KP, off:off + 128]

            # C slabs for chunk mc: cts[k] = [Cr^T | Ci^T] ([126, 1008]),
            # cds[k] = (Cr-Ci)^T ([126, 504]) formed on-device.
            cts = {}
            cds = {}

            def load_c(k, mc):
                eng = nc.sync if (k % 2 == 0) else nc.scalar
                ct = cp.tile([128, 2 * NW], F32R, tag="cc")
                eng.dma_start(out=ct[:KP, :], in_=CC[k, mc])
                cd = dp.tile([128, NW], F32R, tag="cd")
                nc.vector.tensor_sub(cd[:KP, :], ct[:KP, 0:NW], ct[:KP, NW:2 * NW])
                cts[(k, mc)] = ct
                cds[(k, mc)] = cd

            # Interleaved initial loads: X slab k, C slab (k, mc=0) so the
            # PE can start after the first pair lands.
            for k in range(KT):
                eng = nc.sync if (k % 2 == 0) else nc.scalar
                eng.dma_start(
                    out=xt[:KP, k * 2 * ROWS:(k + 1) * 2 * ROWS], in_=XX[k]
                )
                nc.vector.tensor_add(
                    out=xs[:KP, k * ROWS:(k + 1) * ROWS],
                    in0=xt[:KP, k * 2 * ROWS:k * 2 * ROWS + ROWS],
                    in1=xt[:KP, k * 2 * ROWS + ROWS:(k + 1) * 2 * ROWS],
                )
                load_c(k, 0)

            for mc in range(NCH):
                if mc > 0:
                    for k in range(KT):
                        load_c(k, mc)
                for n in range(NT):
                    m1 = pp.tile([128, NW], F32, tag="m1")
                    m2 = pp.tile([128, NW], F32, tag="m2")
                    m3 = pp.tile([128, NW], F32, tag="m3")
                    for k in range(KT):
                        cr = cts[(k, mc)][:KP, 0:NW]
                        ci = cts[(k, mc)][:KP, NW:2 * NW]
                        cd = cds[(k, mc)][:KP, :]
                        st, sp = (k == 0), (k == KT - 1)
                        nc.tensor.matmul(m1[:], xsl(k, 0, n), cr, start=st, stop=sp)
                        nc.tensor.matmul(m2[:], xsl(k, 1, n), ci, start=st, stop=sp)
                        nc.tensor.matmul(m3[:], xssl(k, n), cd, start=st, stop=sp)
                    # Yr = m1 + m2 ; Yi = m3 - m1 + m2
                    a = op.tile([128, NW], F32, tag="a")
                    yr = op.tile([128, NW], F32, tag="yr")
                    d = op.tile([128, NW], F32, tag="d")
                    yi = op.tile([128, NW], F32, tag="yi")
                    nc.vector.tensor_copy(out=a[:], in_=m1[:])
                    nc.vector.tensor_add(out=yr[:], in0=a[:], in1=m2[:])
                    nc.vector.tensor_sub(d[:], m3[:], a[:])
                    nc.vector.tensor_add(out=yi[:], in0=d[:], in1=m2[:])
                    rsl = slice(n * 128, (n + 1) * 128)
                    csl = slice(mc * NW, (mc + 1) * NW)
                    nc.scalar.dma_start(out=YR[rsl, csl], in_=yr[:])
                    nc.sync.dma_start(out=YI[rsl, csl], in_=yi[:])
    nc.compile()
    return nc


def _get_nc():
    global _NC
    if _NC is None:
        _NC = _build_program()
    return _NC


def kernel(x_real, x_imag, C_real, C_imag, sc_ind):
    global LAST_RESULTS
    xr = np.asarray(x_real, dtype=np.float32)
    xi = np.asarray(x_imag, dtype=np.float32)
    cr = np.asarray(C_real, dtype=np.float32)
    ci = np.asarray(C_imag, dtype=np.float32)
    sc = np.asarray(sc_ind)

    # Host prep: gather effective subcarriers, flatten, transpose.
    idx = sc.astype(np.int64)
    xgr = xr[..., idx].reshape(B * T * S, L)      # [2048, 1512]
    xgi = xi[..., idx].reshape(B * T * S, L)
    xrT = np.ascontiguousarray(xgr.T)             # [1512, 2048]
    xiT = np.ascontiguousarray(xgi.T)

    # C^T slabs, r/i concatenated: CC[k, mc, p, 0:504]=Cr^T, [504:]=Ci^T
    crT = cr.T.reshape(KT, KP, NCH, NW)
    ciT = ci.T.reshape(KT, KP, NCH, NW)
    CC = np.empty((KT, NCH, KP, 2 * NW), dtype=np.float32)
    CC[..., 0:NW] = crT.transpose(0, 2, 1, 3)
    CC[..., NW:] = ciT.transpose(0, 2, 1, 3)
    CC = np.ascontiguousarray(CC)

    in_maps = []
    for c in range(NCORES):
        cols = slice(c * ROWS, (c + 1) * ROWS)
        XXc = np.empty((KT, KP, 2 * ROWS), dtype=np.float32)
        XXc[..., 0:ROWS] = xrT[:, cols].reshape(KT, KP, ROWS)
        XXc[..., ROWS:] = xiT[:, cols].reshape(KT, KP, ROWS)
        in_maps.append({"XX": np.ascontiguousarray(XXc), "CC": CC})

    nc = _get_nc()
    res = run_bass_kernel_spmd(
        nc, in_maps, core_ids=list(range(NCORES)), trace=TRACE
    )
    LAST_RESULTS = res

    yr_full = np.concatenate([r["YR"] for r in res.results], axis=0)
    yi_full = np.concatenate([r["YI"] for r in res.results], axis=0)

    out = np.zeros((2, B, T, S, SYM, FFT), dtype=np.float32)
    out[0].reshape(B * T * S, SYM, FFT)[:, :, idx] = yr_full.reshape(
        B * T * S, SYM, NSC
    )
    out[1].reshape(B * T * S, SYM, FFT)[:, :, idx] = yi_full.reshape(
        B * T * S, SYM, NSC
    )
    return out


# revision 7
# speedup vs baseline: 1.0356x; 1.0356x over previous
"""DeMash kernel for Trainium2 (8 NeuronCores, Bass/Tile).

Math: Y = X @ C^H over rows n = (B,T,S) flattened, with a subcarrier
gather before and scatter after. Complex multiply via the 3-matmul
Gauss trick (all operands f32r -> full PE rate, ~1.5e-4 rel err):
    m1 = Xr @ Cr^T,  m2 = Xi @ Ci^T,  m3 = (Xr+Xi) @ (Cr-Ci)^T
    Yr = m1 + m2,    Yi = m3 - m1 + m2
Sharding: data-parallel over batch (axis 0), 32 batches -> 256 rows per
core; C replicated. Stationary = X^T tiles, moving = C^T slabs, PSUM
accumulation over the L=1512 contraction in 12 tiles of 126. Xs and Cd
are formed on-device (DVE) to keep HBM traffic at the fp32 minimum.
"""

import numpy as np
import concourse.bass as bass
import concourse.mybir as mybir
from concourse import bacc
from concourse.tile import TileContext
from concourse.bass_utils import run_bass_kernel_spmd

B, T, S, SYM, FFT = 256, 4, 2, 14, 128
NSC = 108
L = SYM * NSC                   # 1512
NCORES = 8
ROWS = (B // NCORES) * T * S    # 256 rows per core
NT = ROWS // 128                # 2 row tiles of 128
KT, KP = 12, 126                # contraction tiles: 12 x 126 = 1512
NCH, NW = 3, 504                # output column chunks: 3 x 504 = 1512

F32 = mybir.dt.float32
F32R = mybir.dt.float32r

TRACE = False                   # test harness flips this for profiling
LAST_RESULTS = None             # stashed BassKernelResults for the harness

_NC = None


def _build_program():
    nc = bacc.Bacc("TRN2", target_bir_lowering=False, debug=False)
    XX = nc.dram_tensor("XX", [KT, KP, 2 * ROWS], F32R, kind="ExternalInput")
    CC = nc.dram_tensor("CC", [KT, NCH, KP, 2 * NW], F32R, kind="ExternalInput")
    YR = nc.dram_tensor("YR", [ROWS, L], F32, kind="ExternalOutput")
    YI = nc.dram_tensor("YI", [ROWS, L], F32, kind="ExternalOutput")

    with TileContext(nc) as tc:
        with (
            tc.tile_pool(name="xp", bufs=1) as xp,
            tc.tile_pool(name="cp", bufs=26) as cp,
            tc.tile_pool(name="op", bufs=6) as op,
            tc.tile_pool(name="pp", bufs=2, space="PSUM") as pp,
        ):
            # Resident stationary operands: X^T [126, (k, ri, n)] and
            # Xn = -Xr^T for the imaginary accumulation.
            xt = xp.tile([128, KT * 2 * ROWS], F32R, tag="xt")
            xn = xp.tile([128, KT * ROWS], F32R, tag="xn")

            def xsl(k, ri, n):
                off = k * 2 * ROWS + ri * ROWS + n * 128
                return xt[:KP, off:off + 128]

            def xnsl(k, n):
                off = k * ROWS + n * 128
                return xn[:KP, off:off + 128]

            # C slabs for chunk mc: cts[(k, mc)] = [Cr^T | Ci^T] [126, 1008]
            cts = {}

            def load_c(k, mc):
                eng = nc.sync if (k % 2 == 0) else nc.scalar
                ct = cp.tile([128, 2 * NW], F32R, tag="cc")
                eng.dma_start(out=ct[:KP, :], in_=CC[k, mc])
                cts[(k, mc)] = ct

            # Interleaved initial loads: X slab k, C slab (k, mc=0) so the
            # PE can start after the first pair lands.
            for k in range(KT):
                eng = nc.scalar if (k % 2 == 0) else nc.sync
                eng.dma_start(
                    out=xt[:KP, k * 2 * ROWS:(k + 1) * 2 * ROWS], in_=XX[k]
                )
                nc.vector.tensor_scalar_mul(
                    xn[:KP, k * ROWS:(k + 1) * ROWS],
                    xt[:KP, k * 2 * ROWS:k * 2 * ROWS + ROWS],
                    -1.0,
                )
                load_c(k, 0)

            for mc in range(NCH):
                if mc > 0:
                    for k in range(KT):
                        load_c(k, mc)
                for n in range(NT):
                    pr = pp.tile([128, NW], F32, tag="pr")
                    pi = pp.tile([128, NW], F32, tag="pi")
                    for k in range(KT):
                        cr = cts[(k, mc)][:KP, 0:NW]
                        ci = cts[(k, mc)][:KP, NW:2 * NW]
                        nc.tensor.matmul(pr[:], xsl(k, 0, n), cr,
                                         start=(k == 0), stop=False)
                        nc.tensor.matmul(pr[:], xsl(k, 1, n), ci,
                                         start=False, stop=(k == KT - 1))
                        nc.tensor.matmul(pi[:], xsl(k, 1, n), cr,
                                         start=(k == 0), stop=False)
                        nc.tensor.matmul(pi[:], xnsl(k, n), ci,
                                         start=False, stop=(k == KT - 1))
                    yr = op.tile([128, NW], F32, tag="yr")
                    yi = op.tile([128, NW], F32, tag="yi")
                    nc.vector.tensor_copy(out=yr[:], in_=pr[:])
                    nc.vector.tensor_copy(out=yi[:], in_=pi[:])
                    rsl = slice(n * 128, (n + 1) * 128)
                    csl = slice(mc * NW, (mc + 1) * NW)
                    nc.scalar.dma_start(out=YR[rsl, csl], in_=yr[:])
                    nc.sync.dma_start(out=YI[rsl, csl], in_=yi[:])
    nc.compile()
    return nc


def _get_nc():
    global _NC
    if _NC is None:
        _NC = _build_program()
    return _NC


def kernel(x_real, x_imag, C_real, C_imag, sc_ind):
    global LAST_RESULTS
    xr = np.asarray(x_real, dtype=np.float32)
    xi = np.asarray(x_imag, dtype=np.float32)
    cr = np.asarray(C_real, dtype=np.float32)
    ci = np.asarray(C_imag, dtype=np.float32)
    sc = np.asarray(sc_ind)

    # Host prep: gather effective subcarriers, flatten, transpose.
    idx = sc.astype(np.int64)
    xgr = xr[..., idx].reshape(B * T * S, L)      # [2048, 1512]
    xgi = xi[..., idx].reshape(B * T * S, L)
    xrT = np.ascontiguousarray(xgr.T)             # [1512, 2048]
    xiT = np.ascontiguousarray(xgi.T)

    # C^T slabs, r/i concatenated: CC[k, mc, p, 0:504]=Cr^T, [504:]=Ci^T
    crT = cr.T.reshape(KT, KP, NCH, NW)
    ciT = ci.T.reshape(KT, KP, NCH, NW)
    CC = np.empty((KT, NCH, KP, 2 * NW), dtype=np.float32)
    CC[..., 0:NW] = crT.transpose(0, 2, 1, 3)
    CC[..., NW:] = ciT.transpose(0, 2, 1, 3)
    CC = np.ascontiguousarray(CC)

    in_maps = []
    for c in range(NCORES):
        cols = slice(c * ROWS, (c + 1) * ROWS)
        XXc = np.empty((KT, KP, 2 * ROWS), dtype=np.float32)
        XXc[..., 0:ROWS] = xrT[:, cols].reshape(KT, KP, ROWS)
        XXc[..., ROWS:] = xiT[:, cols].reshape(KT, KP, ROWS)
        in_maps.append({"XX": np.ascontiguousarray(XXc), "CC": CC})

    nc = _get_nc()
    res = run_bass_kernel_spmd(
        nc, in_maps, core_ids=list(range(NCORES)), trace=TRACE
    )
    LAST_RESULTS = res

    yr_full = np.concatenate([r["YR"] for r in res.results], axis=0)
    yi_full = np.concatenate([r["YI"] for r in res.results], axis=0)

    out = np.zeros((2, B, T, S, SYM, FFT), dtype=np.float32)
    out[0].reshape(B * T * S, SYM, FFT)[:, :, idx] = yr_full.reshape(
        B * T * S, SYM, NSC
    )
    out[1].reshape(B * T * S, SYM, FFT)[:, :, idx] = yi_full.reshape(
        B * T * S, SYM, NSC
    )
    return out
